# revision 1
# baseline (speedup 1.0000x reference)
"""Trainium2 Bass kernel for nn_CopulaDecoder (sparse neighbor attention decoder).

Sharding: data-parallel over batch B=8 -> 8 NeuronCores (one batch item each).
Each core runs the full per-batch forward:
  merged = [encoded | u]                             [V, 257]
  att_value = encoded[pred] @ Wds + bds              [P, 256]
  for l in 0,1:
    keys/values = per-head 3-layer MLPs (bf16 PE)    [V, 8, 32] each
    neighbor attention (dma_gather + DVE)            [P, 256]
    LN -> FF -> LN  (fp32 residual path)
  decoder MLP -> logits [P, 100] -> NLL sum -> loss scalar

All matmuls bf16 (fp32 PSUM accumulation); residual/LN/softmax math fp32.
"""
import sys

sys.path.insert(0, "/opt/trn_rl_repo")

import math
import numpy as np
import ml_dtypes

import concourse.bacc as bacc
import concourse.bass_isa as bass_isa
import concourse.mybir as mybir
import concourse.tile as tile
from concourse.bass_utils import run_bass_kernel_spmd
from concourse.masks import make_identity

F32 = mybir.dt.float32
BF16 = mybir.dt.bfloat16
I16 = mybir.dt.int16

# Problem dims (hardcoded per contract)
B, V, P, N = 8, 4096, 2048, 32
IN_DIM, H, D, L = 256, 8, 32, 2
MLP, RES = 128, 100
HD = H * D          # 256
EPS = 1e-5
SCALE = D ** -0.5   # 1/sqrt(32)
LOG_RES = math.log(RES)

MROW = 256          # merged staging row elems (enc only), 512B bf16 rows
KVROW = 2 * HD      # 512 elems (k all heads | v all heads), 1024B bf16
NT = P // 128       # 16 point tiles
VT = V // 128       # 32 token chunks of 128
TCH = 512           # MLP token chunk
AluOp = None  # set lazily


def _wrap_idx(idx_flat):
    """int16 index array -> dma_gather wrapped layout [128, n/16]."""
    n = idx_flat.shape[0]
    w = idx_flat.reshape(n // 16, 16).T.astype(np.int16)  # [16, n/16]
    return np.tile(w, (8, 1)).copy()                      # [128, n/16]


def build_program(ln_trivial):
    nc = bacc.Bacc()
    op = mybir.AluOpType
    ACTF = mybir.ActivationFunctionType

    # ---------------- DRAM tensors ----------------
    enc = nc.dram_tensor("enc", [V, IN_DIM], F32, kind="ExternalInput")
    uu = nc.dram_tensor("uu", [V, 1], F32, kind="ExternalInput")
    kW1 = nc.dram_tensor("kW1", [L, H, IN_DIM + 1, MLP], F32, kind="ExternalInput")
    kb1 = nc.dram_tensor("kb1", [L, H, MLP], F32, kind="ExternalInput")
    kW2 = nc.dram_tensor("kW2", [L, H, MLP, MLP], F32, kind="ExternalInput")
    kb2 = nc.dram_tensor("kb2", [L, H, MLP], F32, kind="ExternalInput")
    kW3 = nc.dram_tensor("kW3", [L, H, MLP, D], F32, kind="ExternalInput")
    vW1 = nc.dram_tensor("vW1", [L, H, IN_DIM + 1, MLP], F32, kind="ExternalInput")
    vb1 = nc.dram_tensor("vb1", [L, H, MLP], F32, kind="ExternalInput")
    vW2 = nc.dram_tensor("vW2", [L, H, MLP, MLP], F32, kind="ExternalInput")
    vb2 = nc.dram_tensor("vb2", [L, H, MLP], F32, kind="ExternalInput")
    vW3 = nc.dram_tensor("vW3", [L, H, MLP, D], F32, kind="ExternalInput")
    # kb3|vb3 merged on host -> [L, 512] f32
    b3kv = nc.dram_tensor("b3kv", [L, KVROW], F32, kind="ExternalInput")
    Wds = nc.dram_tensor("Wds", [IN_DIM, HD], F32, kind="ExternalInput")
    bds = nc.dram_tensor("bds", [1, HD], F32, kind="ExternalInput")
    fW1 = nc.dram_tensor("fW1", [L, HD, HD], F32, kind="ExternalInput")
    fb1 = nc.dram_tensor("fb1", [L, HD], F32, kind="ExternalInput")
    fW2 = nc.dram_tensor("fW2", [L, HD, HD], F32, kind="ExternalInput")
    fb2 = nc.dram_tensor("fb2", [L, HD], F32, kind="ExternalInput")
    dW1 = nc.dram_tensor("dW1", [HD, MLP], F32, kind="ExternalInput")
    db1 = nc.dram_tensor("db1", [MLP, 1], F32, kind="ExternalInput")
    dW2 = nc.dram_tensor("dW2", [MLP, MLP], F32, kind="ExternalInput")
    db2 = nc.dram_tensor("db2", [MLP, 1], F32, kind="ExternalInput")
    dW3 = nc.dram_tensor("dW3", [MLP, RES], F32, kind="ExternalInput")
    db3 = nc.dram_tensor("db3", [1, RES], F32, kind="ExternalInput")
    ln_g = nc.dram_tensor("ln_g", [2 * L, HD], F32, kind="ExternalInput")  # ln1_0,ln2_0,ln1_1,ln2_1
    ln_b = nc.dram_tensor("ln_b", [2 * L, HD], F32, kind="ExternalInput")
    att_idx = nc.dram_tensor("att_idx", [128, NT * (128 * N // 16)], I16, kind="ExternalInput")
    pred_idx = nc.dram_tensor("pred_idx", [128, P // 16], I16, kind="ExternalInput")
    u_pred = nc.dram_tensor("u_pred", [128, NT], F32, kind="ExternalInput")

    loss_out = nc.dram_tensor("loss_out", [1, 1], F32, kind="ExternalOutput")

    merged = nc.dram_tensor("merged", [V, MROW], BF16)      # staging for X^T + att0 gather
    kvd = [nc.dram_tensor(f"kv{l}", [V, KVROW], BF16) for l in range(L)]

    with tile.TileContext(nc) as tc:
        wc_cm = tc.tile_pool(name="wc", bufs=1)
        wc = wc_cm.__enter__()

        # ---------------- persistent tiles ----------------
        ident = wc.tile([128, 128], F32)
        make_identity(nc, ident[:])
        zeros_bf = wc.tile([128, TCH], BF16)
        nc.vector.memset(zeros_bf[:], 0.0)
        eps_t = wc.tile([128, 1], F32)
        nc.vector.memset(eps_t[:], EPS)
        ones_row = wc.tile([1, 128], BF16)
        nc.vector.memset(ones_row[:], 1.0)
        iota100 = wc.tile([128, RES], F32)
        nc.gpsimd.iota(iota100[:], [[1, RES]], channel_multiplier=0,
                       allow_small_or_imprecise_dtypes=True)

        # ---------------- weights to SBUF (bf16 via SWDGE cast DMA) ----------------
        # per (l, s, h): W1 chunks [128, 2, 128], W1u [1, 128], W2 [128,128], W3 [128,32],
        # b1 [128,1], b2 [128,1]
        W1c, W1u, W2c, W3c, B1c, B2c = {}, {}, {}, {}, {}, {}
        for l in range(L):
            for s, (w1, b1, w2, b2, w3) in enumerate(
                    [(kW1, kb1, kW2, kb2, kW3), (vW1, vb1, vW2, vb2, vW3)]):
                for h in range(H):
                    t = wc.tile([128, 2, 128], BF16, tag=f"W1c{l}{s}{h}")
                    nc.gpsimd.dma_start(
                        out=t[:], in_=w1[l, h, 0:256, :].rearrange("(c p) m -> p c m", p=128))
                    W1c[l, s, h] = t
                    t = wc.tile([1, 128], BF16, tag=f"W1u{l}{s}{h}")
                    nc.gpsimd.dma_start(out=t[:], in_=w1[l, h, 256:257, :])
                    W1u[l, s, h] = t
                    t = wc.tile([128, 128], BF16, tag=f"W2c{l}{s}{h}")
                    nc.gpsimd.dma_start(out=t[:], in_=w2[l, h, :, :])
                    W2c[l, s, h] = t
                    t = wc.tile([128, D], BF16, tag=f"W3c{l}{s}{h}")
                    nc.gpsimd.dma_start(out=t[:], in_=w3[l, h, :, :])
                    W3c[l, s, h] = t
                    t = wc.tile([128, 1], F32, tag=f"B1c{l}{s}{h}")
                    nc.sync.dma_start(out=t[:], in_=b1[l, h, :].unsqueeze(1))
                    B1c[l, s, h] = t
                    t = wc.tile([128, 1], F32, tag=f"B2c{l}{s}{h}")
                    nc.sync.dma_start(out=t[:], in_=b2[l, h, :].unsqueeze(1))
                    B2c[l, s, h] = t
        B3rep = []
        for l in range(L):
            t = wc.tile([128, KVROW], F32, tag=f"B3rep{l}")
            nc.sync.dma_start(out=t[:], in_=b3kv[l:l + 1, :].broadcast_to([128, KVROW]))
            B3rep.append(t)

        WdsC = wc.tile([128, 2, HD], BF16)
        nc.gpsimd.dma_start(out=WdsC[:], in_=Wds[:, :].rearrange("(c p) m -> p c m", p=128))
        BdsRep = wc.tile([128, HD], F32)
        nc.sync.dma_start(out=BdsRep[:], in_=bds[0:1, :].broadcast_to([128, HD]))

        FW1, FW2, FB1, FB2 = [], [], [], []
        for l in range(L):
            t = wc.tile([128, 2, HD], BF16, tag=f"FW1{l}")
            nc.gpsimd.dma_start(out=t[:], in_=fW1[l].rearrange("(c p) m -> p c m", p=128))
            FW1.append(t)
            t = wc.tile([128, 2, HD], BF16, tag=f"FW2{l}")
            nc.gpsimd.dma_start(out=t[:], in_=fW2[l].rearrange("(c p) m -> p c m", p=128))
            FW2.append(t)
            t = wc.tile([128, 2], F32, tag=f"FB1{l}")
            nc.sync.dma_start(out=t[:], in_=fb1[l].rearrange("(c p) -> p c", p=128))
            FB1.append(t)
            t = wc.tile([128, 2], F32, tag=f"FB2{l}")
            nc.sync.dma_start(out=t[:], in_=fb2[l].rearrange("(c p) -> p c", p=128))
            FB2.append(t)
        LNG, LNB = [], []
        if not ln_trivial:
            for i in range(2 * L):
                t = wc.tile([128, HD], F32, tag=f"LNG{i}")
                nc.sync.dma_start(out=t[:], in_=ln_g[i:i + 1, :].broadcast_to([128, HD]))
                LNG.append(t)
                t = wc.tile([128, HD], F32, tag=f"LNB{i}")
                nc.sync.dma_start(out=t[:], in_=ln_b[i:i + 1, :].broadcast_to([128, HD]))
                LNB.append(t)

        DW1 = wc.tile([128, 2, MLP], BF16)
        nc.gpsimd.dma_start(out=DW1[:], in_=dW1[:, :].rearrange("(c p) m -> p c m", p=128))
        DB1 = wc.tile([128, 1], F32)
        nc.sync.dma_start(out=DB1[:], in_=db1[:, :])
        DW2 = wc.tile([128, MLP], BF16)
        nc.gpsimd.dma_start(out=DW2[:], in_=dW2[:, :])
        DB2 = wc.tile([128, 1], F32)
        nc.sync.dma_start(out=DB2[:], in_=db2[:, :])
        DW3 = wc.tile([128, RES], BF16)
        nc.gpsimd.dma_start(out=DW3[:], in_=dW3[:, :])
        DB3 = wc.tile([1, RES], BF16)
        nc.gpsimd.dma_start(out=DB3[:], in_=db3[:, :])

        idx_att = wc.tile([128, NT * (128 * N // 16)], I16)
        nc.sync.dma_start(out=idx_att[:], in_=att_idx[:, :])
        idx_pred = wc.tile([128, P // 16], I16)
        nc.sync.dma_start(out=idx_pred[:], in_=pred_idx[:, :])

        # persistent activations
        att_value = wc.tile([128, NT, HD], F32)      # token-major residual stream
        att_acc = wc.tile([128, NT, HD], F32)        # attention output accumulator
        q_bf = wc.tile([128, NT, HD], BF16)
        u_keep = wc.tile([128, NT], F32)             # u at pred points
        loss_acc = wc.tile([128, 1], F32)
        nc.vector.memset(loss_acc[:], 0.0)

        # ---------------- Phase 0: merged staging + X^T ----------------
        nc.gpsimd.dma_start(
            out=merged[:, :].rearrange("(c p) e -> p c e", p=128),
            in_=enc[:, :].rearrange("(c p) e -> p c e", p=128))

        with tc.tile_pool(name="p1", bufs=1) as p1:
            XT = []
            for c in range(2):
                t = p1.tile([128, V], BF16, tag=f"XT{c}")
                nc.sync.dma_start_transpose(out=t[:], in_=merged[:, c * 128:(c + 1) * 128])
                XT.append(t)
            t = p1.tile([1, V], BF16, tag="XT2")
            nc.gpsimd.dma_start(out=t[:], in_=uu[:, :].rearrange("v e -> e v"))
            XT.append(t)

            # ---------------- Phase 1: keys/values MLPs -> kv DRAM ----------------
            with (
                tc.tile_pool(name="mlp", bufs=2) as pm,
                tc.tile_pool(name="ps1", bufs=2, space="PSUM") as ps1,
                tc.tile_pool(name="ps3", bufs=2, space="PSUM") as ps3,
            ):
                for l in range(L):
                    for tch in range(V // TCH):
                        tsl = slice(tch * TCH, (tch + 1) * TCH)
                        h2s = {}
                        for s in range(2):
                            for h in range(H):
                                pm1 = ps1.tile([128, TCH], F32, tag="pm1")
                                nc.tensor.matmul(pm1[:], W1c[l, s, h][:, 0, :],
                                                 XT[0][:, tsl], start=True, stop=False)
                                nc.tensor.matmul(pm1[:], W1c[l, s, h][:, 1, :],
                                                 XT[1][:, tsl], start=False, stop=False)
                                nc.tensor.matmul(pm1[:], W1u[l, s, h][:],
                                                 XT[2][0:1, tsl], start=False, stop=True)
                                h1 = pm.tile([128, TCH], BF16, tag="h1")
                                if (h + s) % 2 == 0:
                                    nc.scalar.activation(out=h1[:], in_=pm1[:], func=ACTF.Relu,
                                                         bias=B1c[l, s, h][:], scale=1.0)
                                else:
                                    nc.vector.scalar_tensor_tensor(
                                        out=h1[:], in0=pm1[:], scalar=B1c[l, s, h][:],
                                        in1=zeros_bf[:], op0=op.add, op1=op.max)
                                pm2 = ps1.tile([128, TCH], F32, tag="pm2")
                                nc.tensor.matmul(pm2[:], W2c[l, s, h][:], h1[:],
                                                 start=True, stop=True)
                                h2 = pm.tile([128, TCH], BF16, tag=f"h2_{s}_{h}")
                                if (h + s) % 2 == 1:
                                    nc.scalar.activation(out=h2[:], in_=pm2[:], func=ACTF.Relu,
                                                         bias=B2c[l, s, h][:], scale=1.0)
                                else:
                                    nc.vector.scalar_tensor_tensor(
                                        out=h2[:], in0=pm2[:], scalar=B2c[l, s, h][:],
                                        in1=zeros_bf[:], op0=op.add, op1=op.max)
                                h2s[s, h] = h2
                        # L3: token-major [128tok, 512] = [k 8x32 | v 8x32] per 128-token block
                        for m in range(TCH // 128):
                            pkv = ps3.tile([128, KVROW], F32, tag="pkv")
                            msl = slice(m * 128, (m + 1) * 128)
                            first = True
                            for s in range(2):
                                for h in range(H):
                                    nc.tensor.matmul(
                                        pkv[:, s * HD + h * D: s * HD + (h + 1) * D],
                                        h2s[s, h][:, msl], W3c[l, s, h][:],
                                        start=True, stop=True, skip_group_check=not first)
                                    first = False
                            stg = pm.tile([128, KVROW], BF16, tag="stg")
                            nc.vector.tensor_tensor(out=stg[:], in0=pkv[:], in1=B3rep[l][:],
                                                    op=op.add)
                            row0 = tch * TCH + m * 128
                            nc.gpsimd.dma_start(
                                out=kvd[l][row0:row0 + 128, :].rearrange("(c p) e -> p c e", p=128),
                                in_=stg[:].unsqueeze(1))

            # ---------------- att0: gather pred rows of merged, @ Wds ----------------
            with (
                tc.tile_pool(name="a0", bufs=1) as pa0,
                tc.tile_pool(name="psa", bufs=2, space="PSUM") as psa,
            ):
                Xg = pa0.tile([128, NT, MROW], BF16)
                nc.gpsimd.dma_gather(Xg[:], merged[:, :], idx_pred[:],
                                     num_idxs=P, num_idxs_reg=P, elem_size=MROW,
                                     single_packet=False)
                nc.sync.dma_start(out=u_keep[:], in_=u_pred[:, :])
                for t in range(NT):
                    xg32 = pa0.tile([128, IN_DIM], F32, tag="xg32")
                    nc.vector.tensor_copy(out=xg32[:], in_=Xg[:, t, 0:IN_DIM])
                    xgT = pa0.tile([128, 2, 128], BF16, tag="xgT")
                    for c in range(2):
                        ptr = psa.tile([128, 128], F32, tag="ptr")
                        nc.tensor.transpose(ptr[:], xg32[:, c * 128:(c + 1) * 128], ident[:])
                        nc.scalar.copy(out=xgT[:, c, :], in_=ptr[:])
                    pa = psa.tile([128, HD], F32, tag="pa")
                    for c in range(2):
                        nc.tensor.matmul(pa[:], xgT[:, c, :], WdsC[:, c, :],
                                         start=(c == 0), stop=(c == 1))
                    nc.vector.tensor_tensor(out=att_value[:, t, :], in0=pa[:],
                                            in1=BdsRep[:], op=op.add)

            # ---------------- Layers: attention + FF ----------------
            for l in range(L):
                # q = bf16(att_value)
                for t in range(NT):
                    nc.vector.tensor_copy(out=q_bf[:, t, :], in_=att_value[:, t, :])

                with (
                    tc.tile_pool(name=f"att{l}", bufs=1) as pat,
                    tc.tile_pool(name=f"attg{l}", bufs=1) as patg,
                ):
                    ICH = 128 * N // 16  # idx cols per tile
                    for t in range(NT):
                        G = patg.tile([128, N, KVROW], BF16, tag="G")
                        nc.gpsimd.dma_gather(
                            G[:], kvd[l][:, :], idx_att[:, t * ICH:(t + 1) * ICH],
                            num_idxs=128 * N, num_idxs_reg=128 * N, elem_size=KVROW,
                            single_packet=False)
                        # scores
                        Es = pat.tile([128, N, H, D], BF16, tag="Es")
                        qv = q_bf[:, t, :].rearrange("p (h d) -> p h d", h=H) \
                            .unsqueeze(1).broadcast_to([128, N, H, D])
                        nc.vector.tensor_tensor(
                            out=Es[:], in0=G[:, :, 0:HD].rearrange("p n (h d) -> p n h d", h=H),
                            in1=qv, op=op.mult)
                        sc = pat.tile([128, N, H], F32, tag="sc")
                        nc.vector.tensor_reduce(out=sc[:], in_=Es[:],
                                                axis=mybir.AxisListType.X, op=op.add)
                        # softmax over n (stride-H groups)
                        mx = pat.tile([128, H], F32, tag="mx")
                        nc.vector.tensor_reduce(
                            out=mx[:], in_=sc[:].rearrange("p n h -> p h n"),
                            axis=mybir.AxisListType.X, op=op.max)
                        s2 = pat.tile([128, N, H], BF16, tag="s2")
                        nc.vector.tensor_tensor(
                            out=s2[:], in0=sc[:],
                            in1=mx[:].unsqueeze(1).broadcast_to([128, N, H]),
                            op=op.subtract)
                        wE = pat.tile([128, N, H], BF16, tag="wE")
                        nc.scalar.activation(out=wE[:], in_=s2[:], func=ACTF.Exp,
                                             bias=0.0, scale=SCALE)
                        den = pat.tile([128, H], F32, tag="den")
                        nc.vector.tensor_reduce(
                            out=den[:], in_=wE[:].rearrange("p n h -> p h n"),
                            axis=mybir.AxisListType.X, op=op.add)
                        rden = pat.tile([128, H], F32, tag="rden")
                        nc.vector.reciprocal(out=rden[:], in_=den[:])
                        # att = sum_n w * V
                        Ev = Es  # reuse scratch
                        nc.vector.tensor_tensor(
                            out=Ev[:], in0=G[:, :, HD:].rearrange("p n (h d) -> p n h d", h=H),
                            in1=wE[:].unsqueeze(3).broadcast_to([128, N, H, D]),
                            op=op.mult)
                        t1 = pat.tile([128, N // 2, H, D], BF16, tag="t1")
                        nc.vector.tensor_tensor(out=t1[:], in0=Ev[:, 0:16], in1=Ev[:, 16:32],
                                                op=op.add)
                        t2 = pat.tile([128, N // 4, H, D], BF16, tag="t2")
                        nc.vector.tensor_tensor(out=t2[:], in0=t1[:, 0:8], in1=t1[:, 8:16],
                                                op=op.add)
                        t3 = pat.tile([128, N // 8, H, D], BF16, tag="t3")
                        nc.vector.tensor_tensor(out=t3[:], in0=t2[:, 0:4], in1=t2[:, 4:8],
                                                op=op.add)
                        t4 = pat.tile([128, N // 16, H, D], BF16, tag="t4")
                        nc.vector.tensor_tensor(out=t4[:], in0=t3[:, 0:2], in1=t3[:, 2:4],
                                                op=op.add)
                        au = pat.tile([128, H, D], F32, tag="au")
                        nc.vector.tensor_tensor(out=au[:], in0=t4[:, 0], in1=t4[:, 1],
                                                op=op.add)
                        nc.vector.tensor_tensor(
                            out=att_acc[:, t, :].rearrange("p (h d) -> p h d", h=H),
                            in0=au[:],
                            in1=rden[:].unsqueeze(2).broadcast_to([128, H, D]),
                            op=op.mult)

                # residual + ln1 -> att_value ; build xT for FF
                with (
                    tc.tile_pool(name=f"ff{l}", bufs=1) as pf,
                    tc.tile_pool(name=f"psf{l}", bufs=2, space="PSUM") as psf,
                ):
                    xT = pf.tile([128, 2, P], BF16, tag="xT")
                    for t in range(NT):
                        x = pf.tile([128, HD], F32, tag="x")
                        nc.vector.tensor_tensor(out=x[:], in0=att_value[:, t, :],
                                                in1=att_acc[:, t, :], op=op.add)
                        self_ln(nc, tc, pf, x, eps_t,
                                None if ln_trivial else LNG[2 * l],
                                None if ln_trivial else LNB[2 * l], op, ACTF)
                        nc.vector.tensor_copy(out=att_value[:, t, :], in_=x[:])
                        for c in range(2):
                            ptx = psf.tile([128, 128], F32, tag="ptx")
                            nc.tensor.transpose(ptx[:], x[:, c * 128:(c + 1) * 128], ident[:])
                            nc.scalar.copy(out=xT[:, c, t * 128:(t + 1) * 128], in_=ptx[:])
                    # ff1: feature-major out [2x128, P]
                    hT = pf.tile([128, 2, P], BF16, tag="hT")
                    for mch in range(2):
                        for nch in range(P // TCH):
                            nsl = slice(nch * TCH, (nch + 1) * TCH)
                            pff = psf.tile([128, TCH], F32, tag="pff")
                            for c in range(2):
                                nc.tensor.matmul(
                                    pff[:], FW1[l][:, c, mch * 128:(mch + 1) * 128],
                                    xT[:, c, nsl], start=(c == 0), stop=(c == 1))
                            nc.scalar.activation(out=hT[:, mch, nsl], in_=pff[:],
                                                 func=ACTF.Relu,
                                                 bias=FB1[l][:, mch:mch + 1], scale=1.0)
                    foT = pf.tile([128, 2, P], F32, tag="foT")
                    for mch in range(2):
                        for nch in range(P // TCH):
                            nsl = slice(nch * TCH, (nch + 1) * TCH)
                            pff = psf.tile([128, TCH], F32, tag="pff")
                            for c in range(2):
                                nc.tensor.matmul(
                                    pff[:], FW2[l][:, c, mch * 128:(mch + 1) * 128],
                                    hT[:, c, nsl], start=(c == 0), stop=(c == 1))
                            nc.vector.scalar_tensor_tensor(
                                out=foT[:, mch, nsl], in0=pff[:],
                                scalar=FB2[l][:, mch:mch + 1], in1=zeros_bf[:],
                                op0=op.add, op1=op.bypass)
                    # transpose ff out back to token-major, residual add, ln2
                    for t in range(NT):
                        x2 = pf.tile([128, HD], F32, tag="x2")
                        for c in range(2):
                            ptb = psf.tile([128, 128], F32, tag="ptb")
                            nc.tensor.transpose(ptb[:], foT[:, c, t * 128:(t + 1) * 128],
                                                ident[:])
                            nc.vector.tensor_tensor(
                                out=x2[:, c * 128:(c + 1) * 128], in0=ptb[:],
                                in1=att_value[:, t, c * 128:(c + 1) * 128], op=op.add)
                        self_ln(nc, tc, pf, x2, eps_t,
                                None if ln_trivial else LNG[2 * l + 1],
                                None if ln_trivial else LNB[2 * l + 1], op, ACTF)
                        nc.vector.tensor_copy(out=att_value[:, t, :], in_=x2[:])

            # ---------------- decoder + NLL ----------------
            with (
                tc.tile_pool(name="dec", bufs=1) as pd,
                tc.tile_pool(name="psd", bufs=2, space="PSUM") as psd,
            ):
                xT = pd.tile([128, 2, P], BF16, tag="dxT")
                for t in range(NT):
                    for c in range(2):
                        ptx = psd.tile([128, 128], F32, tag="dptx")
                        nc.tensor.transpose(ptx[:], att_value[:, t, c * 128:(c + 1) * 128],
                                            ident[:])
                        nc.scalar.copy(out=xT[:, c, t * 128:(t + 1) * 128], in_=ptx[:])
                h1T = pd.tile([128, P], BF16, tag="h1T")
                for nch in range(P // TCH):
                    nsl = slice(nch * TCH, (nch + 1) * TCH)
                    pp = psd.tile([128, TCH], F32, tag="pp")
                    for c in range(2):
                        nc.tensor.matmul(pp[:], DW1[:, c, :], xT[:, c, nsl],
                                         start=(c == 0), stop=(c == 1))
                    nc.scalar.activation(out=h1T[:, nsl], in_=pp[:], func=ACTF.Relu,
                                         bias=DB1[:], scale=1.0)
                h2T = pd.tile([128, P], BF16, tag="h2T")
                for nch in range(P // TCH):
                    nsl = slice(nch * TCH, (nch + 1) * TCH)
                    pp = psd.tile([128, TCH], F32, tag="pp")
                    nc.tensor.matmul(pp[:], DW2[:], h1T[:, nsl], start=True, stop=True)
                    nc.scalar.activation(out=h2T[:, nsl], in_=pp[:], func=ACTF.Relu,
                                         bias=DB2[:], scale=1.0)
                for t in range(NT):
                    pl = psd.tile([128, RES], F32, tag="pl")
                    nc.tensor.matmul(pl[:], h2T[:, t * 128:(t + 1) * 128], DW3[:],
                                     start=True, stop=False)
                    nc.tensor.matmul(pl[:], ones_row[:], DB3[:], start=False, stop=True)
                    Lg = pd.tile([128, RES], F32, tag="Lg")
                    nc.vector.tensor_copy(out=Lg[:], in_=pl[:])
                    mx = pd.tile([128, 1], F32, tag="dmx")
                    nc.vector.tensor_reduce(out=mx[:], in_=Lg[:],
                                            axis=mybir.AxisListType.X, op=op.max)
                    nmx = pd.tile([128, 1], F32, tag="dnmx")
                    nc.vector.tensor_scalar_mul(out=nmx[:], in0=mx[:], scalar1=-1.0)
                    escr = pd.tile([128, RES], F32, tag="escr")
                    sume = pd.tile([128, 1], F32, tag="sume")
                    nc.scalar.activation(out=escr[:], in_=Lg[:], func=ACTF.Exp,
                                         bias=nmx[:], scale=1.0, accum_out=sume[:])
                    lse = pd.tile([128, 1], F32, tag="lse")
                    nc.scalar.activation(out=lse[:], in_=sume[:], func=ACTF.Ln,
                                         bias=0.0, scale=1.0)
                    # init = mx + lse - log(RES)
                    init = pd.tile([128, 1], F32, tag="init")
                    nc.vector.tensor_tensor(out=init[:], in0=mx[:], in1=lse[:], op=op.add)
                    nc.vector.tensor_scalar_add(out=init[:], in0=init[:], scalar1=-LOG_RES)
                    # negated one-hot: noh = [iota <= us-1] - [iota <= us]
                    us = pd.tile([128, 1], F32, tag="us")
                    nc.vector.tensor_scalar_mul(out=us[:], in0=u_keep[:, t:t + 1],
                                                scalar1=float(RES))
                    us1 = pd.tile([128, 1], F32, tag="us1")
                    nc.vector.tensor_scalar_add(out=us1[:], in0=us[:], scalar1=-1.0)
                    A = pd.tile([128, RES], F32, tag="A")
                    nc.vector.tensor_scalar(out=A[:], in0=iota100[:], scalar1=us[:],
                                            scalar2=None, op0=op.is_le, op1=op.bypass)
                    Bm = pd.tile([128, RES], F32, tag="Bm")
                    nc.vector.tensor_scalar(out=Bm[:], in0=iota100[:], scalar1=us1[:],
                                            scalar2=None, op0=op.is_le, op1=op.bypass)
                    noh = pd.tile([128, RES], F32, tag="noh")
                    nc.vector.tensor_tensor(out=noh[:], in0=Bm[:], in1=A[:], op=op.subtract)
                    scr = pd.tile([128, RES], F32, tag="scr")
                    tls = pd.tile([128, 1], F32, tag="tls")
                    nc.vector.tensor_tensor(out=scr[:], in0=Lg[:], in1=noh[:], op=op.mult)
                    nc.vector.tensor_reduce(out=tls[:], in_=scr[:],
                                            axis=mybir.AxisListType.X, op=op.add)
                    nc.vector.tensor_tensor(out=tls[:], in0=tls[:], in1=init[:], op=op.add)
                    nc.vector.tensor_tensor(out=loss_acc[:], in0=loss_acc[:], in1=tls[:],
                                            op=op.add)
                # partition sum -> scalar
                lsum = pd.tile([128, 1], F32, tag="lsum")
                nc.gpsimd.partition_all_reduce(lsum[:], loss_acc[:], channels=128,
                                               reduce_op=bass_isa.ReduceOp.add)
                nc.gpsimd.dma_start(out=loss_out[:, :], in_=lsum[0:1, :])

        wc_cm.__exit__(None, None, None)

    nc.compile()
    return nc


def self_ln(nc, tc, pool, x, eps_t, g_rep, b_rep, op, ACTF):
    """In-place layernorm over free dim (256) of x [128, 256] f32."""
    st = pool.tile([128, 6], F32, tag="ln_st")
    nc.vector.bn_stats(out=st[:], in_=x[:])
    mv = pool.tile([128, 2], F32, tag="ln_mv")
    nc.vector.bn_aggr(out=mv[:], in_=st[:])
    rstd = pool.tile([128, 1], F32, tag="ln_rstd")
    nc.scalar.activation(out=rstd[:], in_=mv[:, 1:2], func=ACTF.Sqrt,
                         bias=eps_t[:], scale=1.0)
    nc.vector.reciprocal(out=rstd[:], in_=rstd[:])
    nc.vector.tensor_scalar(out=x[:], in0=x[:], scalar1=mv[:, 0:1], scalar2=rstd[:],
                            op0=op.subtract, op1=op.mult)
    if g_rep is not None:
        nc.vector.tensor_tensor(out=x[:], in0=x[:], in1=g_rep[:], op=op.mult)
        nc.vector.tensor_tensor(out=x[:], in0=x[:], in1=b_rep[:], op=op.add)


_prog_cache = {}
last_exec_time_ns = None
last_trace_path = None


def kernel(**inputs):
    inp = {k: np.asarray(v) for k, v in inputs.items()}
    enc = np.ascontiguousarray(inp["encoded"], dtype=np.float32)      # [B, V, 256]
    uu = np.ascontiguousarray(inp["true_u"], dtype=np.float32)        # [B, V]
    pred = np.asarray(inp["pred_points"]).astype(np.int64)            # [P]
    nb = np.asarray(inp["neighbor_index"]).astype(np.int64)           # [P, N]

    ln_trivial = all(
        np.all(inp[k] == 1.0) for k in ("ln1_g", "ln2_g")) and all(
        np.all(inp[k] == 0.0) for k in ("ln1_b", "ln2_b"))

    # host-built gather indices
    # att gather: per point-tile t, i = n*128 + q -> nb[t*128+q, n]
    att_list = []
    for t in range(NT):
        idx = np.empty(128 * N, np.int64)
        for n in range(N):
            idx[n * 128:(n + 1) * 128] = nb[t * 128:(t + 1) * 128, n]
        att_list.append(_wrap_idx(idx))
    att_idx = np.concatenate(att_list, axis=1).astype(np.int16)       # [128, NT*256]
    pred_idx = _wrap_idx(pred)                                        # [128, 128]

    b3kv = np.concatenate([inp["kb3"].reshape(L, HD), inp["vb3"].reshape(L, HD)],
                          axis=1).astype(np.float32)                  # [L, 512]
    ln_g = np.stack([inp["ln1_g"][0], inp["ln2_g"][0],
                     inp["ln1_g"][1], inp["ln2_g"][1]]).astype(np.float32)
    ln_b = np.stack([inp["ln1_b"][0], inp["ln2_b"][0],
                     inp["ln1_b"][1], inp["ln2_b"][1]]).astype(np.float32)

    shared = {
        "kW1": inp["kW1"].astype(np.float32), "kb1": inp["kb1"].astype(np.float32),
        "kW2": inp["kW2"].astype(np.float32), "kb2": inp["kb2"].astype(np.float32),
        "kW3": inp["kW3"].astype(np.float32),
        "vW1": inp["vW1"].astype(np.float32), "vb1": inp["vb1"].astype(np.float32),
        "vW2": inp["vW2"].astype(np.float32), "vb2": inp["vb2"].astype(np.float32),
        "vW3": inp["vW3"].astype(np.float32),
        "b3kv": b3kv,
        "Wds": inp["Wds"].astype(np.float32), "bds": inp["bds"].reshape(1, HD).astype(np.float32),
        "fW1": inp["fW1"].astype(np.float32), "fb1": inp["fb1"].astype(np.float32),
        "fW2": inp["fW2"].astype(np.float32), "fb2": inp["fb2"].astype(np.float32),
        "dW1": inp["dW1"].astype(np.float32), "db1": inp["db1"].reshape(MLP, 1).astype(np.float32),
        "dW2": inp["dW2"].astype(np.float32), "db2": inp["db2"].reshape(MLP, 1).astype(np.float32),
        "dW3": inp["dW3"].astype(np.float32), "db3": inp["db3"].reshape(1, RES).astype(np.float32),
        "ln_g": ln_g, "ln_b": ln_b,
        "att_idx": att_idx, "pred_idx": pred_idx,
    }

    in_maps = []
    for b in range(B):
        m = dict(shared)
        m["enc"] = np.ascontiguousarray(enc[b])
        m["uu"] = np.ascontiguousarray(uu[b].reshape(V, 1))
        m["u_pred"] = np.ascontiguousarray(
            uu[b][pred].reshape(NT, 128).T.astype(np.float32))
        in_maps.append(m)

    key = ("prog", ln_trivial)
    if key not in _prog_cache:
        _prog_cache[key] = build_program(ln_trivial)
    nc = _prog_cache[key]

    import os
    trace = os.environ.get("BASS_TRACE", "0") == "1"
    res = run_bass_kernel_spmd(nc, in_maps, core_ids=list(range(B)), trace=trace)
    global last_exec_time_ns, last_trace_path
    last_exec_time_ns = res.exec_time_ns
    last_trace_path = res.instructions_and_trace[1] if res.instructions_and_trace else None
    out = np.array([res.results[b]["loss_out"][0, 0] for b in range(B)], dtype=np.float32)
    return out



# revision 2
# speedup vs baseline: 1.0116x; 1.0116x over previous
"""Trainium2 Bass kernel for nn_CopulaDecoder — v2 (DVE-diet redesign).

Sharding: data-parallel over batch B=8 -> 8 NeuronCores.

Key ideas vs v1:
  - residual-stream features stored in (d,h) order so every big attention
    elementwise op (score mult, d-tree, value mult, n-tree) runs in the DVE
    2x perf mode (all-bf16, packed inner dim).
  - kv rows in DRAM are [k(d,h) | v(d,h)]; neighbor gather [128, N, 2, D, H].
  - no softmax max-subtraction (scores are tiny); rsqrt/recip via exp/ln on
    Act so the whole kernel stays in one activation-table set.
  - q0 = enc@Wds computed densely token-major (bias via ones-row matmul),
    staged to DRAM bf16, row-gathered at pred points.
  - layer-1 K/V MLP interleaved into layer-0's attention tile loop; relus on
    Act (and Pool when overlapped with attention).
  - batched weight loads (one cast-DMA per tensor).
"""
import sys

sys.path.insert(0, "/opt/trn_rl_repo")

import math
import numpy as np

import concourse.bacc as bacc
import concourse.bass_isa as bass_isa
import concourse.mybir as mybir
import concourse.tile as tile
from concourse.bass_utils import run_bass_kernel_spmd
from concourse.masks import make_identity

F32 = mybir.dt.float32
BF16 = mybir.dt.bfloat16
I16 = mybir.dt.int16
I32T = mybir.dt.int32

B, V, P, N = 8, 4096, 2048, 32
IN_DIM, H, D, L = 256, 8, 32, 2
MLP, RES = 128, 100
HD = H * D
EPS = 1e-5
SCALE = D ** -0.5
LOG_RES = math.log(RES)

KVROW = 2 * HD
NT = P // 128
VT = V // 128
TCH = 512
ICH = 128 * N // 16  # idx cols per point-tile in wrapped layout


def _wrap_idx(idx_flat):
    n = idx_flat.shape[0]
    w = idx_flat.reshape(n // 16, 16).T.astype(np.int16)
    return np.tile(w, (8, 1)).copy()


def build_program(ln_trivial):
    nc = bacc.Bacc()
    op = mybir.AluOpType
    ACTF = mybir.ActivationFunctionType
    X = mybir.AxisListType.X

    # ---------------- DRAM tensors ----------------
    enc = nc.dram_tensor("enc", [V, IN_DIM], F32, kind="ExternalInput")
    uu = nc.dram_tensor("uu", [V, 1], F32, kind="ExternalInput")
    # per-head MLP weights, batched layouts (see host prep):
    # w1a: [128, L*S*H*C, MLP] rows (c p) of W1[:256]; w1u: [1, L*S*H, MLP]
    w1a = nc.dram_tensor("w1a", [128, L * 2 * H * 2 * MLP], F32, kind="ExternalInput")
    w1u = nc.dram_tensor("w1u", [1, L * 2 * H * MLP], F32, kind="ExternalInput")
    w2a = nc.dram_tensor("w2a", [128, L * 2 * H * MLP], F32, kind="ExternalInput")
    w3a = nc.dram_tensor("w3a", [128, L * 2 * H * D], F32, kind="ExternalInput")
    b1a = nc.dram_tensor("b1a", [128, L * 2 * H], F32, kind="ExternalInput")
    b2a = nc.dram_tensor("b2a", [128, L * 2 * H], F32, kind="ExternalInput")
    b3kv = nc.dram_tensor("b3kv", [L, KVROW], F32, kind="ExternalInput")  # (d,h) perm
    Wds = nc.dram_tensor("Wds", [IN_DIM, HD], F32, kind="ExternalInput")  # cols perm
    bds = nc.dram_tensor("bds", [1, HD], F32, kind="ExternalInput")       # perm
    fW1 = nc.dram_tensor("fW1", [L, HD, HD], F32, kind="ExternalInput")   # both perm
    fb1 = nc.dram_tensor("fb1", [L, HD], F32, kind="ExternalInput")
    fW2 = nc.dram_tensor("fW2", [L, HD, HD], F32, kind="ExternalInput")
    fb2 = nc.dram_tensor("fb2", [L, HD], F32, kind="ExternalInput")
    dW1 = nc.dram_tensor("dW1", [HD, MLP], F32, kind="ExternalInput")     # rows perm
    db1 = nc.dram_tensor("db1", [MLP, 1], F32, kind="ExternalInput")
    dW2 = nc.dram_tensor("dW2", [MLP, MLP], F32, kind="ExternalInput")
    db2 = nc.dram_tensor("db2", [MLP, 1], F32, kind="ExternalInput")
    dW3 = nc.dram_tensor("dW3", [MLP, RES], F32, kind="ExternalInput")
    db3 = nc.dram_tensor("db3", [1, RES], F32, kind="ExternalInput")
    ln_g = nc.dram_tensor("ln_g", [2 * L, HD], F32, kind="ExternalInput")  # perm
    ln_b = nc.dram_tensor("ln_b", [2 * L, HD], F32, kind="ExternalInput")
    att_idx = nc.dram_tensor("att_idx", [128, NT * ICH], I16, kind="ExternalInput")
    pred_idx = nc.dram_tensor("pred_idx", [128, P // 16], I16, kind="ExternalInput")
    u_pred = nc.dram_tensor("u_pred", [128, NT], F32, kind="ExternalInput")

    loss_out = nc.dram_tensor("loss_out", [1, 1], F32, kind="ExternalOutput")

    merged = nc.dram_tensor("merged", [V, IN_DIM], BF16)
    q0d = nc.dram_tensor("q0d", [V, HD], BF16)
    kvd = [nc.dram_tensor(f"kv{l}", [V, KVROW], BF16) for l in range(L)]

    with tile.TileContext(nc) as tc:
        wc_cm = tc.tile_pool(name="wc", bufs=1)
        wc = wc_cm.__enter__()

        # ---------------- persistent small tiles ----------------
        ident = wc.tile([128, 128], F32)
        make_identity(nc, ident[:])
        zeros_bf = wc.tile([128, TCH], BF16)
        nc.vector.memset(zeros_bf[:], 0.0)
        eps_t = wc.tile([128, 1], F32)
        nc.vector.memset(eps_t[:], EPS)
        ones_row = wc.tile([1, 128], BF16)
        nc.vector.memset(ones_row[:], 1.0)
        iota100 = wc.tile([128, RES], F32)
        nc.gpsimd.iota(iota100[:], [[1, RES]], channel_multiplier=0,
                       allow_small_or_imprecise_dtypes=True)

        # ---------------- staging: merged bf16 + X^T ----------------
        nc.gpsimd.dma_start(
            out=merged[:, :].rearrange("(c p) e -> p c e", p=128),
            in_=enc[:, :].rearrange("(c p) e -> p c e", p=128))

        XT = wc.tile([128, 2, V], BF16)
        for c in range(2):
            nc.sync.dma_start_transpose(out=XT[:, c, :], in_=merged[:, c * 128:(c + 1) * 128])
        XU = wc.tile([1, V], BF16)
        nc.gpsimd.dma_start(out=XU[:], in_=uu[:, :].rearrange("v e -> e v"))

        # ---------------- batched weight loads ----------------
        # per-head MLP weights: tile [128, l, s, h, c, m] etc.
        W1 = wc.tile([128, L, 2, H, 2, MLP], BF16)
        nc.gpsimd.dma_start(
            out=W1[:], in_=w1a[:, :].rearrange(
                "p (l s h c m) -> p l s h c m", l=L, s=2, h=H, c=2))
        W1U = wc.tile([1, L, 2, H, MLP], BF16)
        nc.gpsimd.dma_start(
            out=W1U[:], in_=w1u[:, :].rearrange("p (l s h m) -> p l s h m", l=L, s=2, h=H))
        W2 = wc.tile([128, L, 2, H, MLP], BF16)
        nc.gpsimd.dma_start(
            out=W2[:], in_=w2a[:, :].rearrange("p (l s h m) -> p l s h m", l=L, s=2, h=H))
        W3 = wc.tile([128, L, 2, H, D], BF16)
        nc.gpsimd.dma_start(
            out=W3[:], in_=w3a[:, :].rearrange("p (l s h m) -> p l s h m", l=L, s=2, h=H))
        B1 = wc.tile([128, L, 2, H], F32)
        nc.sync.dma_start(
            out=B1[:], in_=b1a[:, :].rearrange("p (l s h) -> p l s h", l=L, s=2))
        B2 = wc.tile([128, L, 2, H], F32)
        nc.sync.dma_start(
            out=B2[:], in_=b2a[:, :].rearrange("p (l s h) -> p l s h", l=L, s=2))
        B3rep = []
        for l in range(L):
            t = wc.tile([128, KVROW], F32, tag=f"B3rep{l}")
            nc.sync.dma_start(out=t[:], in_=b3kv[l:l + 1, :].broadcast_to([128, KVROW]))
            B3rep.append(t)

        WdsC = wc.tile([128, 2, HD], BF16)
        nc.gpsimd.dma_start(out=WdsC[:], in_=Wds[:, :].rearrange("(c p) m -> p c m", p=128))
        BdsB = wc.tile([1, HD], BF16)
        nc.gpsimd.dma_start(out=BdsB[:], in_=bds[:, :])

        FW1 = wc.tile([128, L, 2, HD], BF16)
        nc.gpsimd.dma_start(out=FW1[:], in_=fW1[:, :, :].rearrange(
            "l (c p) m -> p l c m", p=128))
        FW2 = wc.tile([128, L, 2, HD], BF16)
        nc.gpsimd.dma_start(out=FW2[:], in_=fW2[:, :, :].rearrange(
            "l (c p) m -> p l c m", p=128))
        FB1 = wc.tile([128, L, 2], F32)
        nc.sync.dma_start(out=FB1[:], in_=fb1[:, :].rearrange("l (c p) -> p l c", p=128))
        FB2 = wc.tile([128, L, 2], F32)
        nc.sync.dma_start(out=FB2[:], in_=fb2[:, :].rearrange("l (c p) -> p l c", p=128))
        LNG, LNB = [], []
        if not ln_trivial:
            for i in range(2 * L):
                t = wc.tile([128, HD], F32, tag=f"LNG{i}")
                nc.sync.dma_start(out=t[:], in_=ln_g[i:i + 1, :].broadcast_to([128, HD]))
                LNG.append(t)
                t = wc.tile([128, HD], F32, tag=f"LNB{i}")
                nc.sync.dma_start(out=t[:], in_=ln_b[i:i + 1, :].broadcast_to([128, HD]))
                LNB.append(t)

        DW1 = wc.tile([128, 2, MLP], BF16)
        nc.gpsimd.dma_start(out=DW1[:], in_=dW1[:, :].rearrange("(c p) m -> p c m", p=128))
        DB1 = wc.tile([128, 1], F32)
        nc.sync.dma_start(out=DB1[:], in_=db1[:, :])
        DW2 = wc.tile([128, MLP], BF16)
        nc.gpsimd.dma_start(out=DW2[:], in_=dW2[:, :])
        DB2 = wc.tile([128, 1], F32)
        nc.sync.dma_start(out=DB2[:], in_=db2[:, :])
        DW3 = wc.tile([128, RES], BF16)
        nc.gpsimd.dma_start(out=DW3[:], in_=dW3[:, :])
        DB3 = wc.tile([1, RES], BF16)
        nc.gpsimd.dma_start(out=DB3[:], in_=db3[:, :])

        idx_att = wc.tile([128, NT * ICH], I16)
        nc.sync.dma_start(out=idx_att[:], in_=att_idx[:, :])
        idx_pred = wc.tile([128, P // 16], I16)
        nc.sync.dma_start(out=idx_pred[:], in_=pred_idx[:, :])

        att_value = wc.tile([128, NT, HD], F32)
        u_keep = wc.tile([128, NT], F32)
        nc.sync.dma_start(out=u_keep[:], in_=u_pred[:, :])
        loss_acc = wc.tile([128, 1], F32)
        nc.vector.memset(loss_acc[:], 0.0)

        MAGIC = 0x5F3759DF

        def ln_stats(pool, xap, mv_all, t):
            st = pool.tile([128, 6], F32, tag="ln_st")
            nc.vector.bn_stats(out=st[:], in_=xap)
            nc.vector.bn_aggr(out=mv_all[:, t, :], in_=st[:])

        def rsqrt_batch(pool, mv_all, n):
            """rstd[:, 0:n] = (var + EPS) ** -0.5 via Newton, DVE-only."""
            ve = pool.tile([128, NT], F32, tag="rs_ve")
            nc.vector.tensor_scalar(out=ve[:, 0:n], in0=mv_all[:, 0:n, 1],
                                    scalar1=EPS, scalar2=None,
                                    op0=op.add, op1=op.bypass)
            sh = pool.tile([128, NT], I32T, tag="rs_sh")
            nc.vector.tensor_scalar(out=sh[:, 0:n], in0=ve[:, 0:n].bitcast(I32T),
                                    scalar1=1, scalar2=None,
                                    op0=op.arith_shift_right, op1=op.bypass)
            nc.vector.tensor_scalar(out=sh[:, 0:n], in0=sh[:, 0:n], scalar1=-1,
                                    scalar2=MAGIC, op0=op.mult, op1=op.add)
            yy = pool.tile([128, NT], F32, tag="rs_yy")
            nc.vector.tensor_copy(out=yy[:, 0:n], in_=sh[:, 0:n].bitcast(F32))
            t1 = pool.tile([128, NT], F32, tag="rs_t1")
            for _ in range(2):
                nc.vector.tensor_tensor(out=t1[:, 0:n], in0=yy[:, 0:n],
                                        in1=yy[:, 0:n], op=op.mult)
                nc.vector.tensor_tensor(out=t1[:, 0:n], in0=t1[:, 0:n],
                                        in1=ve[:, 0:n], op=op.mult)
                nc.vector.tensor_scalar(out=t1[:, 0:n], in0=t1[:, 0:n], scalar1=-0.5,
                                        scalar2=1.5, op0=op.mult, op1=op.add)
                nc.vector.tensor_tensor(out=yy[:, 0:n], in0=yy[:, 0:n],
                                        in1=t1[:, 0:n], op=op.mult)
            return yy

        def ln_apply(xap, mv_all, rstd, t, gi):
            nc.vector.tensor_scalar(out=xap, in0=xap, scalar1=mv_all[:, t, 0:1],
                                    scalar2=rstd[:, t:t + 1],
                                    op0=op.subtract, op1=op.mult)
            if not ln_trivial:
                nc.vector.tensor_tensor(out=xap, in0=xap, in1=LNG[gi][:], op=op.mult)
                nc.vector.tensor_tensor(out=xap, in0=xap, in1=LNB[gi][:], op=op.add)

        def ln_pass(pool, gi):
            mv_all = pool.tile([128, NT, 2], F32, tag="ln_mv_all")
            for t in range(NT):
                ln_stats(pool, att_value[:, t, :], mv_all, t)
            rstd = rsqrt_batch(pool, mv_all, NT)
            for t in range(NT):
                ln_apply(att_value[:, t, :], mv_all, rstd, t, gi)

        # ---------------- q0 = enc@Wds + bds (token-major), stage + gather ----
        with (
            tc.tile_pool(name="q0", bufs=2) as pq0,
            tc.tile_pool(name="psq", bufs=2, space="PSUM") as psq,
        ):
            for vb in range(VT):
                vsl = slice(vb * 128, (vb + 1) * 128)
                pq = psq.tile([128, HD], F32, tag="pq")
                for c in range(2):
                    nc.tensor.matmul(pq[:], XT[:, c, vsl], WdsC[:, c, :],
                                     start=(c == 0), stop=False)
                nc.tensor.matmul(pq[:], ones_row[:], BdsB[:], start=False, stop=True)
                stq = pq0.tile([128, HD], BF16, tag="stq")
                nc.scalar.copy(out=stq[:], in_=pq[:])
                row0 = vb * 128
                nc.sync.dma_start(
                    out=q0d[row0:row0 + 128, :].rearrange("(c p) e -> p c e", p=128),
                    in_=stq[:].unsqueeze(1))

        with tc.tile_pool(name="qg", bufs=1) as pqg:
            Qg = pqg.tile([128, NT, HD], BF16)
            nc.gpsimd.dma_gather(Qg[:], q0d[:, :], idx_pred[:],
                                 num_idxs=P, num_idxs_reg=P, elem_size=HD,
                                 single_packet=False)
            nc.scalar.copy(out=att_value[:], in_=Qg[:])

            # decoder NLL mask precompute (fills DVE slack before attention)
            noh_all = wc.tile([128, NT, RES], F32)
            with tc.tile_pool(name="nohp", bufs=2) as pn:
                for t in range(NT):
                    us = pn.tile([128, 1], F32, tag="us")
                    nc.vector.tensor_scalar_mul(out=us[:], in0=u_keep[:, t:t + 1],
                                                scalar1=float(RES))
                    us1 = pn.tile([128, 1], F32, tag="us1")
                    nc.vector.tensor_scalar_add(out=us1[:], in0=us[:], scalar1=-1.0)
                    A = pn.tile([128, RES], F32, tag="A")
                    nc.vector.tensor_scalar(out=A[:], in0=iota100[:], scalar1=us[:],
                                            scalar2=None, op0=op.is_le, op1=op.bypass)
                    Bm = pn.tile([128, RES], F32, tag="Bm")
                    nc.vector.tensor_scalar(out=Bm[:], in0=iota100[:], scalar1=us1[:],
                                            scalar2=None, op0=op.is_le, op1=op.bypass)
                    nc.vector.tensor_tensor(out=noh_all[:, t, :], in0=Bm[:], in1=A[:],
                                            op=op.subtract)

        # ---------------- K/V MLP chunk ----------------
        def mlp_chunk(pm, pmh, ps1, ps3, l, tch, relu1, relu2, stg_eng):
            """One TCH-token chunk of the per-head K/V MLPs for layer l."""
            tsl = slice(tch * TCH, (tch + 1) * TCH)
            h2s = {}

            def w1_stage(s, h):
                pm1 = ps1.tile([128, TCH], F32, tag="pm1")
                nc.tensor.matmul(pm1[:], W1[:, l, s, h, 0, :], XT[:, 0, tsl],
                                 start=True, stop=False)
                nc.tensor.matmul(pm1[:], W1[:, l, s, h, 1, :], XT[:, 1, tsl],
                                 start=False, stop=False)
                nc.tensor.matmul(pm1[:], W1U[:, l, s, h, :], XU[0:1, tsl],
                                 start=False, stop=True)
                return pm1

            def w2_stage(s, h, pm1):
                h1 = pm.tile([128, TCH], BF16, tag="h1")
                relu1(h1, pm1, B1[:, l, s, h:h + 1])
                pm2 = ps1.tile([128, TCH], F32, tag="pm2")
                nc.tensor.matmul(pm2[:], W2[:, l, s, h, :], h1[:],
                                 start=True, stop=True)
                h2 = pmh.tile([128, TCH], BF16, tag=f"h2_{s}_{h}")
                relu2[(s * H + h) % len(relu2)](h2, pm2, B2[:, l, s, h:h + 1])
                h2s[s, h] = h2

            pairs = [(s, h) for s in range(2) for h in range(H)]
            prev = None
            for s, h in pairs:
                pm1 = w1_stage(s, h)
                if prev is not None:
                    w2_stage(*prev)
                prev = (s, h, pm1)
            w2_stage(*prev)
            for m in range(TCH // 128):
                pkv = ps3.tile([128, 2, H, D], F32, tag="pkv")
                msl = slice(m * 128, (m + 1) * 128)
                first = True
                for s in range(2):
                    for h in range(H):
                        nc.tensor.matmul(
                            pkv[:, s, h, :], h2s[s, h][:, msl], W3[:, l, s, h, :],
                            start=True, stop=True, skip_group_check=not first)
                        first = False
                stg = pm.tile([128, KVROW], BF16, tag="stg")
                stg_eng.tensor_tensor(
                    out=stg[:].rearrange("p (s d h) -> p s d h", s=2, h=H),
                    in0=pkv[:].rearrange("p s h d -> p s d h"),
                    in1=B3rep[l][:].rearrange("p (s d h) -> p s d h", s=2, h=H),
                    op=op.add)
                row0 = tch * TCH + m * 128
                nc.sync.dma_start(
                    out=kvd[l][row0:row0 + 128, :].rearrange("(c p) e -> p c e", p=128),
                    in_=stg[:].unsqueeze(1))

        def relu_act(o, i, b):
            nc.scalar.activation(out=o[:], in_=i[:], func=ACTF.Relu, bias=b, scale=1.0)

        def relu_dve(o, i, b):
            nc.vector.scalar_tensor_tensor(out=o[:], in0=i[:], scalar=b,
                                           in1=zeros_bf[:], op0=op.add, op1=op.max)

        def relu_pool(o, i, b):
            nc.gpsimd.scalar_tensor_tensor(out=o[:], in0=i[:], scalar=b,
                                           in1=zeros_bf[:], op0=op.add, op1=op.max)

        # ---------------- attention tile ----------------
        def att_tile(patg, pat, l, t):
            G = patg.tile([128, N, KVROW], BF16, tag="G")
            nc.gpsimd.dma_gather(
                G[:], kvd[l][:, :], idx_att[:, t * ICH:(t + 1) * ICH],
                num_idxs=128 * N, num_idxs_reg=128 * N, elem_size=KVROW,
                single_packet=False)
            Gv = G[:].rearrange("p n (s d h) -> p n s d h", s=2, h=H)
            qt = pat.tile([128, D, H], BF16, tag="qt")
            nc.scalar.copy(
                out=qt[:], in_=att_value[:, t, :].rearrange("p (d h) -> p d h", h=H))
            Es = pat.tile([128, N, D, H], BF16, tag="Es")
            nc.vector.tensor_tensor(
                out=Es[:], in0=Gv[:, :, 0, :, :],
                in1=qt[:].unsqueeze(1).broadcast_to([128, N, D, H]), op=op.mult)
            # in-place d-halving tree: Es[:, :, 0:k, :] += Es[:, :, k:2k, :]
            for k in (16, 8, 4, 2):
                nc.vector.tensor_tensor(out=Es[:, :, 0:k, :], in0=Es[:, :, 0:k, :],
                                        in1=Es[:, :, k:2 * k, :], op=op.add)
            sc = pat.tile([128, N, H], BF16, tag="sc")
            nc.vector.tensor_tensor(out=sc[:], in0=Es[:, :, 0, :],
                                    in1=Es[:, :, 1, :], op=op.add)
            wE = pat.tile([128, N, H], BF16, tag="wE")
            nc.scalar.activation(out=wE[:], in_=sc[:], func=ACTF.Exp,
                                 bias=0.0, scale=SCALE)
            den = pat.tile([128, H], F32, tag="den")
            nc.vector.tensor_reduce(out=den[:], in_=wE[:].rearrange("p n h -> p h n"),
                                    axis=X, op=op.add)
            rden = pat.tile([128, H], F32, tag="rden")
            nc.vector.reciprocal(out=rden[:], in_=den[:])
            wn = pat.tile([128, N, H], BF16, tag="wn")
            nc.vector.tensor_tensor(
                out=wn[:], in0=wE[:],
                in1=rden[:].unsqueeze(1).broadcast_to([128, N, H]), op=op.mult)
            nc.vector.tensor_tensor(  # Ev reuses Es storage
                out=Es[:], in0=Gv[:, :, 1, :, :],
                in1=wn[:].unsqueeze(2).broadcast_to([128, N, D, H]), op=op.mult)
            Ev = Es[:].rearrange("p n d h -> p n (d h)")
            # in-place n-halving tree
            for k in (16, 8, 4, 2):
                nc.vector.tensor_tensor(out=Ev[:, 0:k, :], in0=Ev[:, 0:k, :],
                                        in1=Ev[:, k:2 * k, :], op=op.add)
            ab = pat.tile([128, HD], BF16, tag="ab")
            nc.vector.tensor_tensor(out=ab[:], in0=Ev[:, 0, :], in1=Ev[:, 1, :],
                                    op=op.add)
            nc.vector.tensor_tensor(out=att_value[:, t, :], in0=ab[:],
                                    in1=att_value[:, t, :], op=op.add)

        # ---------------- FF block for layer l ----------------
        def ff_block(l):
            with (
                tc.tile_pool(name=f"ff{l}", bufs=1) as pf,
                tc.tile_pool(name=f"psf{l}", bufs=2, space="PSUM") as psf,
            ):
                xT = pf.tile([128, 2, P], BF16, tag="xT")
                for t in range(NT):
                    for c in range(2):
                        ptx = psf.tile([128, 128], F32, tag="ptx")
                        nc.tensor.transpose(ptx[:], att_value[:, t, c * 128:(c + 1) * 128],
                                            ident[:])
                        nc.scalar.copy(out=xT[:, c, t * 128:(t + 1) * 128], in_=ptx[:])
                hT = pf.tile([128, 2, P], BF16, tag="hT")
                for mch in range(2):
                    for nch in range(P // TCH):
                        nsl = slice(nch * TCH, (nch + 1) * TCH)
                        pff = psf.tile([128, TCH], F32, tag="pff")
                        for c in range(2):
                            nc.tensor.matmul(
                                pff[:], FW1[:, l, c, mch * 128:(mch + 1) * 128],
                                xT[:, c, nsl], start=(c == 0), stop=(c == 1))
                        nc.scalar.activation(out=hT[:, mch, nsl], in_=pff[:],
                                             func=ACTF.Relu,
                                             bias=FB1[:, l, mch:mch + 1], scale=1.0)
                foT = pf.tile([128, 2, P], F32, tag="foT")
                for mch in range(2):
                    for nch in range(P // TCH):
                        nsl = slice(nch * TCH, (nch + 1) * TCH)
                        pff = psf.tile([128, TCH], F32, tag="pff")
                        for c in range(2):
                            nc.tensor.matmul(
                                pff[:], FW2[:, l, c, mch * 128:(mch + 1) * 128],
                                hT[:, c, nsl], start=(c == 0), stop=(c == 1))
                        nc.scalar.activation(out=foT[:, mch, nsl], in_=pff[:],
                                             func=ACTF.Identity,
                                             bias=FB2[:, l, mch:mch + 1], scale=1.0)
                mv_all = pf.tile([128, NT, 2], F32, tag="ffmv")
                for t in range(NT):
                    for c in range(2):
                        ptb = psf.tile([128, 128], F32, tag="ptb")
                        nc.tensor.transpose(ptb[:], foT[:, c, t * 128:(t + 1) * 128],
                                            ident[:])
                        nc.vector.tensor_tensor(
                            out=att_value[:, t, c * 128:(c + 1) * 128], in0=ptb[:],
                            in1=att_value[:, t, c * 128:(c + 1) * 128], op=op.add)
                    ln_stats(pf, att_value[:, t, :], mv_all, t)
                rstd = rsqrt_batch(pf, mv_all, NT)
                for t in range(NT):
                    ln_apply(att_value[:, t, :], mv_all, rstd, t, 2 * l + 1)

        # ---------------- main schedule ----------------
        with (
            tc.tile_pool(name="mlp", bufs=2) as pm,
            tc.tile_pool(name="mlph", bufs=1) as pmh,
            tc.tile_pool(name="ps1", bufs=2, space="PSUM") as ps1,
            tc.tile_pool(name="ps3", bufs=3, space="PSUM") as ps3,
        ):
            # layer-0 K/V MLP: relus split Act/DVE/Pool round-robin
            for tch in range(V // TCH):
                mlp_chunk(pm, pmh, ps1, ps3, 0, tch, relu_act, (relu_dve,),
                          nc.vector)
            # layer-0 attention, layer-1 MLP interleaved (relus Act+Pool,
            # staging adds on Pool to keep DVE free)
            with (
                tc.tile_pool(name="attg0", bufs=2) as patg,
                tc.tile_pool(name="att0", bufs=1) as pat,
            ):
                for t in range(NT):
                    att_tile(patg, pat, 0, t)
                    if t % 2 == 1:
                        mlp_chunk(pm, pmh, ps1, ps3, 1, t // 2,
                                  relu_act, (relu_act,), nc.vector)
                ln_pass(pat, 0)
        ff_block(0)
        with (
            tc.tile_pool(name="attg1", bufs=2) as patg,
            tc.tile_pool(name="att1", bufs=1) as pat,
        ):
            for t in range(NT):
                att_tile(patg, pat, 1, t)
            ln_pass(pat, 2)
        ff_block(1)

        # ---------------- decoder + NLL ----------------
        with (
            tc.tile_pool(name="dec", bufs=1) as pd,
            tc.tile_pool(name="psd", bufs=2, space="PSUM") as psd,
        ):
            xT = pd.tile([128, 2, P], BF16, tag="dxT")
            for t in range(NT):
                for c in range(2):
                    ptx = psd.tile([128, 128], F32, tag="dptx")
                    nc.tensor.transpose(ptx[:], att_value[:, t, c * 128:(c + 1) * 128],
                                        ident[:])
                    nc.scalar.copy(out=xT[:, c, t * 128:(t + 1) * 128], in_=ptx[:])
            h1T = pd.tile([128, P], BF16, tag="h1T")
            for nch in range(P // TCH):
                nsl = slice(nch * TCH, (nch + 1) * TCH)
                pp = psd.tile([128, TCH], F32, tag="pp")
                for c in range(2):
                    nc.tensor.matmul(pp[:], DW1[:, c, :], xT[:, c, nsl],
                                     start=(c == 0), stop=(c == 1))
                nc.scalar.activation(out=h1T[:, nsl], in_=pp[:], func=ACTF.Relu,
                                     bias=DB1[:], scale=1.0)
            h2T = pd.tile([128, P], BF16, tag="h2T")
            for nch in range(P // TCH):
                nsl = slice(nch * TCH, (nch + 1) * TCH)
                pp = psd.tile([128, TCH], F32, tag="pp")
                nc.tensor.matmul(pp[:], DW2[:], h1T[:, nsl], start=True, stop=True)
                nc.scalar.activation(out=h2T[:, nsl], in_=pp[:], func=ACTF.Relu,
                                     bias=DB2[:], scale=1.0)
            sume_all = pd.tile([128, NT], F32, tag="sume_all")
            tls_all = pd.tile([128, NT], F32, tag="tls_all")
            for t in range(NT):
                pl = psd.tile([128, RES], F32, tag="pl")
                nc.tensor.matmul(pl[:], h2T[:, t * 128:(t + 1) * 128], DW3[:],
                                 start=True, stop=False)
                nc.tensor.matmul(pl[:], ones_row[:], DB3[:], start=False, stop=True)
                escr = pd.tile([128, RES], F32, tag="escr")
                nc.scalar.activation(out=escr[:], in_=pl[:], func=ACTF.Exp,
                                     bias=0.0, scale=1.0,
                                     accum_out=sume_all[:, t:t + 1])
                scr = pd.tile([128, RES], F32, tag="scr")
                nc.vector.tensor_tensor(out=scr[:], in0=pl[:], in1=noh_all[:, t, :],
                                        op=op.mult)
                nc.vector.tensor_reduce(out=tls_all[:, t:t + 1], in_=scr[:],
                                        axis=X, op=op.add)
            # loss = sum_t tls + ln(sume) - log(RES)
            lnall = pd.tile([128, NT], F32, tag="lnall")
            nc.scalar.activation(out=lnall[:], in_=sume_all[:], func=ACTF.Ln,
                                 bias=0.0, scale=1.0)
            nc.vector.tensor_tensor(out=lnall[:], in0=lnall[:], in1=tls_all[:],
                                    op=op.add)
            nc.vector.tensor_reduce(out=loss_acc[:], in_=lnall[:], axis=X, op=op.add)
            nc.vector.tensor_scalar_add(out=loss_acc[:], in0=loss_acc[:],
                                        scalar1=-LOG_RES * NT)
            lsum = pd.tile([128, 1], F32, tag="lsum")
            nc.gpsimd.partition_all_reduce(lsum[:], loss_acc[:], channels=128,
                                           reduce_op=bass_isa.ReduceOp.add)
            nc.gpsimd.dma_start(out=loss_out[:, :], in_=lsum[0:1, :])

        wc_cm.__exit__(None, None, None)

    nc.compile()
    return nc


# PERM[i_new] = old feature index: new order (d,h), old order (h,d)
PERM = np.array([h * D + d for d in range(D) for h in range(H)], dtype=np.int64)


def _pdh(x):
    """Permute last dim from (h,d) order to (d,h) order."""
    return np.ascontiguousarray(x[..., PERM])


_prog_cache = {}
last_exec_time_ns = None
last_trace_path = None


def kernel(**inputs):
    inp = {k: np.asarray(v) for k, v in inputs.items()}
    enc = np.ascontiguousarray(inp["encoded"], dtype=np.float32)
    uu = np.ascontiguousarray(inp["true_u"], dtype=np.float32)
    pred = np.asarray(inp["pred_points"]).astype(np.int64)
    nb = np.asarray(inp["neighbor_index"]).astype(np.int64)

    ln_trivial = all(
        np.all(inp[k] == 1.0) for k in ("ln1_g", "ln2_g")) and all(
        np.all(inp[k] == 0.0) for k in ("ln1_b", "ln2_b"))

    att_list = []
    for t in range(NT):
        idx = np.empty(128 * N, np.int64)
        for n in range(N):
            idx[n * 128:(n + 1) * 128] = nb[t * 128:(t + 1) * 128, n]
        att_list.append(_wrap_idx(idx))
    att_idx = np.concatenate(att_list, axis=1).astype(np.int16)
    pred_idx = _wrap_idx(pred)

    # batched per-head MLP weight layouts: order (l, s, h)
    def stack_ls(kt, vt):
        return np.stack([inp[kt], inp[vt]], axis=1)  # [L, 2, H, ...]

    W1f = stack_ls("kW1", "vW1").astype(np.float32)  # [L,2,H,257,128]
    w1a = np.ascontiguousarray(
        W1f[:, :, :, :256, :].reshape(L, 2, H, 2, 128, MLP)
        .transpose(4, 0, 1, 2, 3, 5).reshape(128, L * 2 * H * 2 * MLP))
    w1u = np.ascontiguousarray(W1f[:, :, :, 256, :].reshape(1, L * 2 * H * MLP))
    w2a = np.ascontiguousarray(
        stack_ls("kW2", "vW2").astype(np.float32)
        .transpose(3, 0, 1, 2, 4).reshape(128, L * 2 * H * MLP))
    w3a = np.ascontiguousarray(
        stack_ls("kW3", "vW3").astype(np.float32)
        .transpose(3, 0, 1, 2, 4).reshape(128, L * 2 * H * D))
    b1a = np.ascontiguousarray(
        stack_ls("kb1", "vb1").astype(np.float32)
        .transpose(3, 0, 1, 2).reshape(128, L * 2 * H))
    b2a = np.ascontiguousarray(
        stack_ls("kb2", "vb2").astype(np.float32)
        .transpose(3, 0, 1, 2).reshape(128, L * 2 * H))
    # b3 rows in (d,h) order: [k | v]
    b3kv = np.concatenate([_pdh(inp["kb3"].reshape(L, HD)),
                           _pdh(inp["vb3"].reshape(L, HD))],
                          axis=1).astype(np.float32)

    fW1p = np.ascontiguousarray(
        inp["fW1"].astype(np.float32)[:, PERM, :][:, :, PERM])
    fW2p = np.ascontiguousarray(
        inp["fW2"].astype(np.float32)[:, PERM, :][:, :, PERM])
    dW1p = np.ascontiguousarray(inp["dW1"].astype(np.float32)[PERM, :])

    ln_g = _pdh(np.stack([inp["ln1_g"][0], inp["ln2_g"][0],
                          inp["ln1_g"][1], inp["ln2_g"][1]]).astype(np.float32))
    ln_b = _pdh(np.stack([inp["ln1_b"][0], inp["ln2_b"][0],
                          inp["ln1_b"][1], inp["ln2_b"][1]]).astype(np.float32))

    shared = {
        "w1a": w1a, "w1u": w1u, "w2a": w2a, "w3a": w3a, "b1a": b1a, "b2a": b2a,
        "b3kv": b3kv,
        "Wds": _pdh(inp["Wds"].astype(np.float32)),
        "bds": _pdh(inp["bds"].reshape(1, HD).astype(np.float32)),
        "fW1": fW1p, "fb1": _pdh(inp["fb1"].astype(np.float32)),
        "fW2": fW2p, "fb2": _pdh(inp["fb2"].astype(np.float32)),
        "dW1": dW1p, "db1": inp["db1"].reshape(MLP, 1).astype(np.float32),
        "dW2": inp["dW2"].astype(np.float32),
        "db2": inp["db2"].reshape(MLP, 1).astype(np.float32),
        "dW3": inp["dW3"].astype(np.float32),
        "db3": inp["db3"].reshape(1, RES).astype(np.float32),
        "ln_g": ln_g, "ln_b": ln_b,
        "att_idx": att_idx, "pred_idx": pred_idx,
    }

    in_maps = []
    for b in range(B):
        m = dict(shared)
        m["enc"] = np.ascontiguousarray(enc[b])
        m["uu"] = np.ascontiguousarray(uu[b].reshape(V, 1))
        m["u_pred"] = np.ascontiguousarray(
            uu[b][pred].reshape(NT, 128).T.astype(np.float32))
        in_maps.append(m)

    key = ("prog", ln_trivial)
    if key not in _prog_cache:
        _prog_cache[key] = build_program(ln_trivial)
    nc = _prog_cache[key]

    import os
    trace = os.environ.get("BASS_TRACE", "0") == "1"
    res = run_bass_kernel_spmd(nc, in_maps, core_ids=list(range(B)), trace=trace)
    global last_exec_time_ns, last_trace_path
    last_exec_time_ns = res.exec_time_ns
    last_trace_path = res.instructions_and_trace[1] if res.instructions_and_trace else None
    out = np.array([res.results[b]["loss_out"][0, 0] for b in range(B)], dtype=np.float32)
    return out
